# revision 18
# baseline (speedup 1.0000x reference)
"""Negative cross-correlation loss: out = -sum(x * y).

Full inputs x, y: (16, 4000, 512, 1) f32. Data-parallel over the shot axis:
2 shots per core on 8 NeuronCores. The kernel is HBM-bandwidth bound, so the
host casts x, y to fp16 before upload (rel err ~2e-4, far inside the 2e-2
gate) — halving HBM traffic halves device time. Each core streams its
2x4000x512 shard as [128, 16000] fp16 tiles (32 KB contiguous DMA lines, the
largest that still double-buffers in SBUF; x on the SP HWDGE ring, y on the
ACT ring), fuses multiply+per-partition-reduce on the vector engine
(scalar_tensor_tensor accum_out), and writes the [128, n_tiles] f32
partials; host finishes the sum and negates.
"""

import numpy as np

import jax
from jax.experimental.shard_map import shard_map
from jax.sharding import Mesh, NamedSharding, PartitionSpec

import concourse.bacc as bacc
import concourse.mybir as mybir
import concourse.tile as tile
from concourse import bass2jax

N_CORES = 8
P = 128
# Per-core shard: 2 shots * 4000 * 512 * 1 = 4_096_000 elements.
SHARD_ELEMS = 2 * 4000 * 512
TILE_W = 4000
N_TILES = SHARD_ELEMS // (P * TILE_W)  # 8
assert N_TILES * P * TILE_W == SHARD_ELEMS
IN_DT = mybir.dt.float16
IN_NP = np.float16


def _build_nc(
    repeat=1,
    tile_w=16000,
    bufs=2,
    taper=None,
    dense_out=True,
    split_dma=True,
    out_via_act=True,
    split_out=0,
    skip_compute=False,
    skip_dma=False,
    compute_mode="stt",
):
    """Bass kernel for one core. `repeat` re-runs the identical body that many
    times (same data, same result) — used only for wall-clock slope timing."""
    n_tiles = SHARD_ELEMS // (P * tile_w)
    assert n_tiles * P * tile_w == SHARD_ELEMS
    # Tile list: (row_block, col_offset, width). `taper` replaces the last
    # full-width tile with narrower ones so the final DVE op (which can only
    # start after the last DMA lands) is short — shrinks the kernel tail.
    tiles = [(t, 0, tile_w) for t in range(n_tiles)]
    if taper:
        assert sum(taper) == tile_w
        last = tiles.pop()[0]
        off = 0
        for w in taper:
            tiles.append((last, off, w))
            off += w
    nc = bacc.Bacc("TRN2", target_bir_lowering=False, debug=False)
    x = nc.dram_tensor("x", [N_TILES * P, TILE_W], IN_DT, kind="ExternalInput")
    y = nc.dram_tensor("y", [N_TILES * P, TILE_W], IN_DT, kind="ExternalInput")
    out = nc.dram_tensor("out", [P, len(tiles)], mybir.dt.float32, kind="ExternalOutput")

    # The DRAM I/O shape is fixed at [N_TILES*P, TILE_W]; re-view it at the
    # requested tile width (pure elementwise reduction — layout-agnostic).
    def _view(ap):
        if tile_w > TILE_W:
            return ap.rearrange("(r s) c -> r (s c)", s=tile_w // TILE_W)
        if tile_w < TILE_W:
            return ap.rearrange("r (s c) -> (r s) c", c=tile_w)
        return ap

    xa = _view(x.ap())
    ya = _view(y.ap())
    oa = out.ap()

    with tile.TileContext(nc) as tc:
        with (
            tc.tile_pool(name="io", bufs=bufs) as io_pool,
            tc.tile_pool(name="red", bufs=1) as red_pool,
        ):
            acc = red_pool.tile([P, len(tiles)], mybir.dt.float32)
            if skip_compute:
                nc.vector.memset(acc[:], 0.0)
            # Dense 2-byte product scratch keeps the DVE op eligible for the
            # 2x packed perf mode (a stride-0 f32 broadcast out would not be).
            if dense_out or compute_mode == "tt_ts":
                prod = red_pool.tile([P, tile_w], IN_DT)
            else:
                prod = red_pool.tile([P, 1], mybir.dt.float32)
            if compute_mode in ("tt_ts", "tt_only", "act_only", "hybrid"):
                prod2 = red_pool.tile([P, tile_w], IN_DT)
            if skip_dma:
                xt0 = red_pool.tile([P, tile_w], IN_DT)
                yt0 = red_pool.tile([P, tile_w], IN_DT)
                nc.sync.dma_start(out=xt0[:], in_=xa[0:P, 0:tile_w])
                nc.sync.dma_start(out=yt0[:], in_=ya[0:P, 0:tile_w])
            for rep in range(repeat):
                for i, (t, off, w) in enumerate(tiles):
                    if skip_dma:
                        xt, yt = xt0, yt0
                    else:
                        xt = io_pool.tile([P, tile_w], IN_DT, tag="xt")
                        yt = io_pool.tile([P, tile_w], IN_DT, tag="yt")
                    rows = slice(t * P, (t + 1) * P)
                    cols = slice(off, off + w)
                    # x on the SP HWDGE ring; optionally y on the otherwise-
                    # idle ACT ring so two descriptor streams feed the SDMAs.
                    y_dma = nc.scalar if split_dma else nc.sync
                    if not skip_dma:
                        nc.sync.dma_start(out=xt[:, :w], in_=xa[rows, cols])
                        y_dma.dma_start(out=yt[:, :w], in_=ya[rows, cols])
                    if skip_compute:  # timing diagnostic only: wrong output
                        continue
                    if compute_mode == "tt_only":
                        # diagnostic: multiply only, no reduce (wrong output)
                        nc.vector.tensor_tensor(
                            out=prod[:, :w],
                            in0=xt[:, :w],
                            in1=yt[:, :w],
                            op=mybir.AluOpType.mult,
                        )
                        continue
                    if compute_mode == "act_only":
                        # diagnostic: ACT copy+accum straight off xt (wrong
                        # output) — isolates ACT reduce throughput
                        nc.scalar.activation(
                            out=prod[:, :w],
                            in_=xt[:, :w],
                            func=mybir.ActivationFunctionType.Copy,
                            accum_out=acc[:, i : i + 1],
                        )
                        continue
                    if compute_mode == "hybrid":
                        # even tiles: fused STT on DVE; odd tiles: TT multiply
                        # on DVE + copy-accumulate reduce on the idle ACT
                        if i % 2 == 0:
                            nc.vector.scalar_tensor_tensor(
                                out=prod[:, :w],
                                in0=xt[:, :w],
                                scalar=1.0,
                                in1=yt[:, :w],
                                op0=mybir.AluOpType.mult,
                                op1=mybir.AluOpType.mult,
                                accum_out=acc[:, i : i + 1],
                            )
                        else:
                            nc.vector.tensor_tensor(
                                out=prod2[:, :w],
                                in0=xt[:, :w],
                                in1=yt[:, :w],
                                op=mybir.AluOpType.mult,
                            )
                            nc.scalar.activation(
                                out=prod2[:, :w],
                                in_=prod2[:, :w],
                                func=mybir.ActivationFunctionType.Copy,
                                accum_out=acc[:, i : i + 1],
                            )
                        continue
                    if compute_mode == "tt_ts":
                        # TT multiply runs in the DVE 2x packed mode (STT has
                        # no fast uop); tensor_scalar's copy-with-accumulate
                        # then reduces at 4x. 0.75 cycles/elem total vs 1.0.
                        nc.vector.tensor_tensor(
                            out=prod[:, :w],
                            in0=xt[:, :w],
                            in1=yt[:, :w],
                            op=mybir.AluOpType.mult,
                        )
                        nc.vector.tensor_scalar(
                            out=prod2[:, :w],
                            in0=prod[:, :w],
                            scalar1=1.0,
                            scalar2=0.0,
                            op0=mybir.AluOpType.mult,
                            op1=mybir.AluOpType.add,
                            accum_out=acc[:, i : i + 1],
                        )
                    else:
                        # acc[:, i] = sum_w xt*yt (per-partition); prod
                        # absorbs the elementwise product.
                        o = (
                            prod[:, :w]
                            if dense_out
                            else prod.broadcast_to(xt[:, :w].shape)
                        )
                        nc.vector.scalar_tensor_tensor(
                            out=o,
                            in0=xt[:, :w],
                            scalar=1.0,
                            in1=yt[:, :w],
                            op0=mybir.AluOpType.mult,
                            op1=mybir.AluOpType.mult,
                            accum_out=acc[:, i : i + 1],
                        )
            # ship the [P, n_tiles] partials (~6 KB); host finishes the sum
            out_engine = nc.scalar if out_via_act else nc.sync
            if split_out:
                # early chunk hides under the input stream; only the last
                # columns ride the final STT->DMA dependency chain
                k = len(tiles) - split_out
                out_engine.dma_start(out=oa[:, :k], in_=acc[:, :k])
                out_engine.dma_start(out=oa[:, k:], in_=acc[:, k:])
            else:
                out_engine.dma_start(out=oa[:, :], in_=acc[:])

    nc.compile()
    return nc


class Runner:
    """Compiles the per-core Bass kernel once and keeps a cached jitted
    shard_map executable over 8 cores (mirrors bass2jax.run_bass_via_pjrt's
    multi-core path, minus the per-call retrace and host concat)."""

    def __init__(self, repeat=1, n_chained=1, **build_kwargs):
        bass2jax.install_neuronx_cc_hook()
        nc = _build_nc(repeat, **build_kwargs)
        self.nc = nc

        in_names = ["x", "y"]
        out_names = ["out"]
        out_shape = None
        for alloc in nc.m.functions[0].allocations:
            if (
                isinstance(alloc, mybir.MemoryLocationSet)
                and alloc.kind == "ExternalOutput"
            ):
                out_shape = tuple(alloc.tensor_shape)
        assert out_shape is not None
        self.out_shape = out_shape
        out_avals = (jax.core.ShapedArray(out_shape, np.float32),)
        all_in_names = tuple(in_names + out_names + [nc.partition_id_tensor.name])

        def _body(x, y, z):
            # n_chained > 1 (timing only): run the same NEFF k times back to
            # back, threading each exec's output in as the next one's
            # out-buffer operand so the execs can't be deduped or reordered.
            # The slope of wall time over k is the full per-NEFF exec time.
            pid = bass2jax.partition_id_tensor()
            o = z
            for _ in range(n_chained):
                (o,) = bass2jax._bass_exec_p.bind(
                    x,
                    y,
                    o,
                    pid,
                    out_avals=out_avals,
                    in_names=all_in_names,
                    out_names=tuple(out_names),
                    lowering_input_output_aliases=(),
                    sim_require_finite=True,
                    sim_require_nnan=True,
                    nc=nc,
                )
            return (o,)

        devices = jax.devices()[:N_CORES]
        assert len(devices) == N_CORES
        self.mesh = Mesh(np.asarray(devices), ("core",))
        self.sharding = NamedSharding(self.mesh, PartitionSpec("core"))
        in_specs = (PartitionSpec("core"),) * 3
        out_specs = (PartitionSpec("core"),)
        self.fn = jax.jit(
            shard_map(
                _body,
                mesh=self.mesh,
                in_specs=in_specs,
                out_specs=out_specs,
                check_rep=False,
            ),
            donate_argnums=(2,),
            keep_unused=True,
        )

    def __call__(self, x_all, y_all):
        """x_all, y_all: [N_CORES * N_TILES * P, TILE_W] fp16 (host or device).
        Returns the per-core partial sums, one row per core."""
        zeros = np.zeros((N_CORES * self.out_shape[0], *self.out_shape[1:]), np.float32)
        (out,) = self.fn(x_all, y_all, zeros)
        return np.asarray(out).reshape(N_CORES, -1).sum(axis=1, dtype=np.float64)


_RUNNER = None


def _get_runner():
    global _RUNNER
    if _RUNNER is None:
        _RUNNER = Runner()
    return _RUNNER


def prep(a):
    """Full f32 input -> the [8192, TILE_W] fp16 layout the device expects.
    Row-block c is exactly core c's shard (shots 2c, 2c+1)."""
    a = np.asarray(a)
    return np.ascontiguousarray(
        a.reshape(N_CORES * N_TILES * P, TILE_W).astype(IN_NP)
    )


def _run_via_spmd(x, y):
    """Fallback for non-axon containers (real /dev/neuron*): the library's own
    SPMD entrypoint, which picks the native-NRT or PJRT path as appropriate."""
    from concourse.bass_utils import run_bass_kernel_spmd

    rows = N_TILES * P
    nc = _build_nc()
    in_maps = [
        {
            "x": np.ascontiguousarray(x[c * rows : (c + 1) * rows]),
            "y": np.ascontiguousarray(y[c * rows : (c + 1) * rows]),
        }
        for c in range(N_CORES)
    ]
    res = run_bass_kernel_spmd(nc, in_maps, core_ids=list(range(N_CORES)))
    return np.array([np.float64(r["out"].sum()) for r in res.results])


def kernel(x, y, win=None, step=None):
    x = prep(x)
    y = prep(y)
    try:
        runner = _get_runner()
        parts = runner(x, y)
        # Guard against a transient bad exec (seen once cold): re-run and
        # require two matching totals; device time is ~50us so this is cheap.
        parts2 = runner(x, y)
        if not np.isclose(parts.sum(), parts2.sum(), rtol=1e-3, atol=1e-2):
            votes = [parts.sum(), parts2.sum()]
            for _ in range(3):
                votes.append(runner(x, y).sum())
            total = np.median(votes)
        else:
            total = parts.sum()
    except Exception:
        total = _run_via_spmd(x, y).sum()
    return np.float32(-np.float64(total))


# revision 19
# speedup vs baseline: 1.0027x; 1.0027x over previous
"""Negative cross-correlation loss: out = -sum(x * y).

Full inputs x, y: (16, 4000, 512, 1) f32. Data-parallel over the shot axis:
2 shots per core on 8 NeuronCores. The kernel is HBM-bandwidth bound, so the
host casts x, y to fp16 before upload (rel err ~2e-4, far inside the 2e-2
gate) — halving HBM traffic halves device time. Each core streams its
2x4000x512 shard as [128, 16000] fp16 tiles (32 KB contiguous DMA lines, the
largest that still double-buffers in SBUF; x on the SP HWDGE ring, y on the
ACT ring), fuses multiply+per-partition-reduce on the vector engine
(scalar_tensor_tensor accum_out), and writes the [128, n_tiles] f32
partials; host finishes the sum and negates.
"""

import numpy as np

import jax
from jax.experimental.shard_map import shard_map
from jax.sharding import Mesh, NamedSharding, PartitionSpec

import concourse.bacc as bacc
import concourse.mybir as mybir
import concourse.tile as tile
from concourse import bass2jax

N_CORES = 8
P = 128
# Per-core shard: 2 shots * 4000 * 512 * 1 = 4_096_000 elements.
SHARD_ELEMS = 2 * 4000 * 512
TILE_W = 4000
N_TILES = SHARD_ELEMS // (P * TILE_W)  # 8
assert N_TILES * P * TILE_W == SHARD_ELEMS
IN_DT = mybir.dt.float16
IN_NP = np.float16


def _build_nc(
    repeat=1,
    tile_w=16000,
    bufs=2,
    taper=None,
    dense_out=True,
    split_dma=True,
    out_via_act=True,
    split_out=0,
    skip_compute=False,
    skip_dma=False,
    compute_mode="stt",
):
    """Bass kernel for one core. `repeat` re-runs the identical body that many
    times (same data, same result) — used only for wall-clock slope timing."""
    n_tiles = SHARD_ELEMS // (P * tile_w)
    assert n_tiles * P * tile_w == SHARD_ELEMS
    # Tile list: (row_block, col_offset, width). `taper` replaces the last
    # full-width tile with narrower ones so the final DVE op (which can only
    # start after the last DMA lands) is short — shrinks the kernel tail.
    tiles = [(t, 0, tile_w) for t in range(n_tiles)]
    if taper:
        assert sum(taper) == tile_w
        last = tiles.pop()[0]
        off = 0
        for w in taper:
            tiles.append((last, off, w))
            off += w
    nc = bacc.Bacc("TRN2", target_bir_lowering=False, debug=False)
    x = nc.dram_tensor("x", [N_TILES * P, TILE_W], IN_DT, kind="ExternalInput")
    y = nc.dram_tensor("y", [N_TILES * P, TILE_W], IN_DT, kind="ExternalInput")
    out = nc.dram_tensor("out", [P, len(tiles)], mybir.dt.float32, kind="ExternalOutput")

    # The DRAM I/O shape is fixed at [N_TILES*P, TILE_W]; re-view it at the
    # requested tile width (pure elementwise reduction — layout-agnostic).
    def _view(ap):
        if tile_w > TILE_W:
            return ap.rearrange("(r s) c -> r (s c)", s=tile_w // TILE_W)
        if tile_w < TILE_W:
            return ap.rearrange("r (s c) -> (r s) c", c=tile_w)
        return ap

    xa = _view(x.ap())
    ya = _view(y.ap())
    oa = out.ap()

    with tile.TileContext(nc) as tc:
        with (
            tc.tile_pool(name="io", bufs=bufs) as io_pool,
            tc.tile_pool(name="red", bufs=1) as red_pool,
        ):
            acc = red_pool.tile([P, len(tiles)], mybir.dt.float32)
            if skip_compute:
                nc.vector.memset(acc[:], 0.0)
            # Dense 2-byte product scratch: measured faster than a stride-0
            # f32 broadcast out (keeps the op eligible for packed perf modes).
            if dense_out or compute_mode == "tt_ts":
                prod = red_pool.tile([P, tile_w], IN_DT)
            else:
                prod = red_pool.tile([P, 1], mybir.dt.float32)
            if compute_mode in ("tt_ts", "tt_only", "act_only", "hybrid"):
                prod2 = red_pool.tile([P, tile_w], IN_DT)
            if skip_dma:
                xt0 = red_pool.tile([P, tile_w], IN_DT)
                yt0 = red_pool.tile([P, tile_w], IN_DT)
                nc.sync.dma_start(out=xt0[:], in_=xa[0:P, 0:tile_w])
                nc.sync.dma_start(out=yt0[:], in_=ya[0:P, 0:tile_w])
            for rep in range(repeat):
                for i, (t, off, w) in enumerate(tiles):
                    if skip_dma:
                        xt, yt = xt0, yt0
                    else:
                        xt = io_pool.tile([P, tile_w], IN_DT, tag="xt")
                        yt = io_pool.tile([P, tile_w], IN_DT, tag="yt")
                    rows = slice(t * P, (t + 1) * P)
                    cols = slice(off, off + w)
                    # x on the SP HWDGE ring; optionally y on the otherwise-
                    # idle ACT ring so two descriptor streams feed the SDMAs.
                    y_dma = nc.scalar if split_dma else nc.sync
                    if not skip_dma:
                        nc.sync.dma_start(out=xt[:, :w], in_=xa[rows, cols])
                        y_dma.dma_start(out=yt[:, :w], in_=ya[rows, cols])
                    if skip_compute:  # timing diagnostic only: wrong output
                        continue
                    if compute_mode == "tt_only":
                        # diagnostic: multiply only, no reduce (wrong output)
                        nc.vector.tensor_tensor(
                            out=prod[:, :w],
                            in0=xt[:, :w],
                            in1=yt[:, :w],
                            op=mybir.AluOpType.mult,
                        )
                        continue
                    if compute_mode == "act_only":
                        # diagnostic: ACT copy+accum straight off xt (wrong
                        # output) — isolates ACT reduce throughput
                        nc.scalar.activation(
                            out=prod[:, :w],
                            in_=xt[:, :w],
                            func=mybir.ActivationFunctionType.Copy,
                            accum_out=acc[:, i : i + 1],
                        )
                        continue
                    if compute_mode == "hybrid":
                        # even tiles: fused STT on DVE; odd tiles: TT multiply
                        # on DVE + copy-accumulate reduce on the idle ACT
                        if i % 2 == 0:
                            nc.vector.scalar_tensor_tensor(
                                out=prod[:, :w],
                                in0=xt[:, :w],
                                scalar=1.0,
                                in1=yt[:, :w],
                                op0=mybir.AluOpType.mult,
                                op1=mybir.AluOpType.mult,
                                accum_out=acc[:, i : i + 1],
                            )
                        else:
                            nc.vector.tensor_tensor(
                                out=prod2[:, :w],
                                in0=xt[:, :w],
                                in1=yt[:, :w],
                                op=mybir.AluOpType.mult,
                            )
                            nc.scalar.activation(
                                out=prod2[:, :w],
                                in_=prod2[:, :w],
                                func=mybir.ActivationFunctionType.Copy,
                                accum_out=acc[:, i : i + 1],
                            )
                        continue
                    if compute_mode == "tt_ts":
                        # TT multiply runs in the DVE 2x packed mode (STT has
                        # no fast uop); tensor_scalar's copy-with-accumulate
                        # then reduces at 4x. 0.75 cycles/elem total vs 1.0.
                        nc.vector.tensor_tensor(
                            out=prod[:, :w],
                            in0=xt[:, :w],
                            in1=yt[:, :w],
                            op=mybir.AluOpType.mult,
                        )
                        nc.vector.tensor_scalar(
                            out=prod2[:, :w],
                            in0=prod[:, :w],
                            scalar1=1.0,
                            scalar2=0.0,
                            op0=mybir.AluOpType.mult,
                            op1=mybir.AluOpType.add,
                            accum_out=acc[:, i : i + 1],
                        )
                    else:
                        # acc[:, i] = sum_w xt*yt (per-partition); prod
                        # absorbs the elementwise product.
                        o = (
                            prod[:, :w]
                            if dense_out
                            else prod.broadcast_to(xt[:, :w].shape)
                        )
                        nc.vector.scalar_tensor_tensor(
                            out=o,
                            in0=xt[:, :w],
                            scalar=1.0,
                            in1=yt[:, :w],
                            op0=mybir.AluOpType.mult,
                            op1=mybir.AluOpType.mult,
                            accum_out=acc[:, i : i + 1],
                        )
            # ship the [P, n_tiles] partials (~6 KB); host finishes the sum
            out_engine = nc.scalar if out_via_act else nc.sync
            if split_out:
                # early chunk hides under the input stream; only the last
                # columns ride the final STT->DMA dependency chain
                k = len(tiles) - split_out
                out_engine.dma_start(out=oa[:, :k], in_=acc[:, :k])
                out_engine.dma_start(out=oa[:, k:], in_=acc[:, k:])
            else:
                out_engine.dma_start(out=oa[:, :], in_=acc[:])

    nc.compile()
    return nc


class Runner:
    """Compiles the per-core Bass kernel once and keeps a cached jitted
    shard_map executable over 8 cores (mirrors bass2jax.run_bass_via_pjrt's
    multi-core path, minus the per-call retrace and host concat)."""

    def __init__(self, repeat=1, n_chained=1, **build_kwargs):
        bass2jax.install_neuronx_cc_hook()
        nc = _build_nc(repeat, **build_kwargs)
        self.nc = nc

        in_names = ["x", "y"]
        out_names = ["out"]
        out_shape = None
        for alloc in nc.m.functions[0].allocations:
            if (
                isinstance(alloc, mybir.MemoryLocationSet)
                and alloc.kind == "ExternalOutput"
            ):
                out_shape = tuple(alloc.tensor_shape)
        assert out_shape is not None
        self.out_shape = out_shape
        out_avals = (jax.core.ShapedArray(out_shape, np.float32),)
        all_in_names = tuple(in_names + out_names + [nc.partition_id_tensor.name])

        def _body(x, y, z):
            # n_chained > 1 (timing only): run the same NEFF k times back to
            # back, threading each exec's output in as the next one's
            # out-buffer operand so the execs can't be deduped or reordered.
            # The slope of wall time over k is the full per-NEFF exec time.
            pid = bass2jax.partition_id_tensor()
            o = z
            for _ in range(n_chained):
                (o,) = bass2jax._bass_exec_p.bind(
                    x,
                    y,
                    o,
                    pid,
                    out_avals=out_avals,
                    in_names=all_in_names,
                    out_names=tuple(out_names),
                    lowering_input_output_aliases=(),
                    sim_require_finite=True,
                    sim_require_nnan=True,
                    nc=nc,
                )
            return (o,)

        devices = jax.devices()[:N_CORES]
        assert len(devices) == N_CORES
        self.mesh = Mesh(np.asarray(devices), ("core",))
        self.sharding = NamedSharding(self.mesh, PartitionSpec("core"))
        in_specs = (PartitionSpec("core"),) * 3
        out_specs = (PartitionSpec("core"),)
        self.fn = jax.jit(
            shard_map(
                _body,
                mesh=self.mesh,
                in_specs=in_specs,
                out_specs=out_specs,
                check_rep=False,
            ),
            donate_argnums=(2,),
            keep_unused=True,
        )

    def __call__(self, x_all, y_all):
        """x_all, y_all: [N_CORES * N_TILES * P, TILE_W] fp16 (host or device).
        Returns the per-core partial sums, one row per core."""
        zeros = np.zeros((N_CORES * self.out_shape[0], *self.out_shape[1:]), np.float32)
        (out,) = self.fn(x_all, y_all, zeros)
        return np.asarray(out).reshape(N_CORES, -1).sum(axis=1, dtype=np.float64)


_RUNNER = None


def _get_runner():
    global _RUNNER
    if _RUNNER is None:
        _RUNNER = Runner()
    return _RUNNER


def prep(a):
    """Full f32 input -> the [8192, TILE_W] fp16 layout the device expects.
    Row-block c is exactly core c's shard (shots 2c, 2c+1)."""
    a = np.asarray(a)
    return np.ascontiguousarray(
        a.reshape(N_CORES * N_TILES * P, TILE_W).astype(IN_NP)
    )


def _run_via_spmd(x, y):
    """Fallback for non-axon containers (real /dev/neuron*): the library's own
    SPMD entrypoint, which picks the native-NRT or PJRT path as appropriate."""
    from concourse.bass_utils import run_bass_kernel_spmd

    rows = N_TILES * P
    nc = _build_nc()
    in_maps = [
        {
            "x": np.ascontiguousarray(x[c * rows : (c + 1) * rows]),
            "y": np.ascontiguousarray(y[c * rows : (c + 1) * rows]),
        }
        for c in range(N_CORES)
    ]
    res = run_bass_kernel_spmd(nc, in_maps, core_ids=list(range(N_CORES)))
    return np.array([np.float64(r["out"].sum()) for r in res.results])


def kernel(x, y, win=None, step=None):
    x = prep(x)
    y = prep(y)
    try:
        runner = _get_runner()
        parts = runner(x, y)
        # Guard against a transient bad exec (seen once cold): re-run and
        # require two matching totals; device time is ~50us so this is cheap.
        parts2 = runner(x, y)
        if not np.isclose(parts.sum(), parts2.sum(), rtol=1e-3, atol=1e-2):
            votes = [parts.sum(), parts2.sum()]
            for _ in range(3):
                votes.append(runner(x, y).sum())
            total = np.median(votes)
        else:
            total = parts.sum()
    except Exception:
        total = _run_via_spmd(x, y).sum()
    return np.float32(-np.float64(total))
